# revision 1
# baseline (speedup 1.0000x reference)
"""BiLSTM + mean-field CRF on 8 Trainium2 NeuronCores.

Strategy: the single 16384-long sequence is split into 8 contiguous
2048-position core slices (data-parallel across cores). Inside each core the
sequence is further split into 128 lanes of 17 consecutive positions each;
every lane warm-starts K=20 steps early from zero state (LSTM forget gates
~sigmoid(small pre-activations) => state influence decays ~0.5^K, so the
truncation error is far below bf16 noise). Both LSTM directions run as
batched 128-lane recurrences; gates come from one fused PSUM accumulation
[x_t; h_{t-1}] @ [W_ih^T; W_hh^T] + bias (bias via a K=1 matmul). The hidden
state is re-transposed each step with TensorE transpose so it can serve as
the stationary operand of the next step's matmul. Logits are computed per
step with tiny N=32 matmuls from the already-transposed state, scattered to
DRAM in position order, and the CRF (conv kernel as a banded 128x128
Toeplitz matmul, softmax via free-dim reduce in a position-on-partitions
layout) runs on position tiles of 128 at stride 78 whose edges erode 5
positions per iteration.
"""
import sys

sys.path.insert(0, "/opt/trn_rl_repo")

import numpy as np
import ml_dtypes

import concourse.bass as bass
import concourse.bacc as bacc
import concourse.mybir as mybir
from concourse.tile import TileContext
from concourse.bass_utils import run_bass_kernel_spmd

F32 = mybir.dt.float32
BF16 = mybir.dt.bfloat16
AF = mybir.ActivationFunctionType

SEQ, EMB, H, G, C = 16384, 512, 512, 2048, 32
NCORES = 8
K = 20                 # halo warm-up steps
ST = 17                # positions per lane
NL = 128               # lanes
STEPS = K + ST         # 37
WINW = NL * ST         # 2176
XW = K + WINW          # 2196 x-window columns per k-tile
CST, NT = 78, 28       # CRF tile stride / count
CRFW = NT * C          # 896
LOGR = 2304            # logits scratch rows (>= 78*27+64+128)
OUTR = 2240            # output rows per core
FILT, NIT = 11, 5

_CACHE = {}


def _build():
    nc = bacc.Bacc("TRN2", target_bir_lowering=False, debug=False, num_devices=NCORES)

    def din(name, shape, dt=BF16):
        return nc.dram_tensor(name, shape, dt, kind="ExternalInput")

    xtf = din("xtf", [4, 128, XW])
    xtb = din("xtb", [4, 128, XW])
    wf = din("wf", [8, 128, G])
    wb = din("wb", [8, 128, G])
    biasf = din("biasf", [1, G])
    biasb = din("biasb", [1, G])
    wlinf = din("wlinf", [4, 128, C])
    wlinb = din("wlinb", [4, 128, C])
    blin = din("blin", [1, C])
    ones = din("ones", [1, 128])
    ident = din("ident", [128, 128])
    rmat = din("rmat", [128, 128], F32)
    shi = din("shi", [128, 128])
    slo = din("slo", [128, 128])
    mf = din("mf", [128, STEPS], F32)
    mb = din("mb", [128, STEPS], F32)
    valid = din("valid", [128, NT], F32)

    out = nc.dram_tensor("out", [OUTR, C], F32, kind="ExternalOutput")
    logf_d = nc.dram_tensor("logf_d", [LOGR, C], F32)
    logb_d = nc.dram_tensor("logb_d", [LOGR, C], F32)

    with TileContext(nc) as tc:
        with (
            tc.tile_pool(name="consts", bufs=1) as cp,
            tc.tile_pool(name="state", bufs=2) as sp,
        ):
            # ---- load constants/inputs into SBUF ----
            xt_sb, w_sb, bias_sb, wlin_sb, msk_sb = {}, {}, {}, {}, {}
            for d, (xs, ws, bs, wl, ms) in (
                ("f", (xtf, wf, biasf, wlinf, mf)),
                ("b", (xtb, wb, biasb, wlinb, mb)),
            ):
                t = cp.tile([128, 4 * XW], BF16, name=f"xt{d}")
                for k in range(4):
                    nc.sync.dma_start(out=t[:, k * XW:(k + 1) * XW], in_=xs[k])
                xt_sb[d] = t
                t = cp.tile([128, 8 * G], BF16, name=f"w{d}")
                for k in range(8):
                    nc.sync.dma_start(out=t[:, k * G:(k + 1) * G], in_=ws[k])
                w_sb[d] = t
                t = cp.tile([1, G], BF16, name=f"bias{d}")
                nc.sync.dma_start(out=t[:], in_=bs[:])
                bias_sb[d] = t
                t = cp.tile([128, 4 * C], BF16, name=f"wlin{d}")
                for k in range(4):
                    nc.sync.dma_start(out=t[:, k * C:(k + 1) * C], in_=wl[k])
                wlin_sb[d] = t
                t = cp.tile([128, STEPS], F32, name=f"msk{d}")
                nc.sync.dma_start(out=t[:], in_=ms[:])
                msk_sb[d] = t

            blin_sb = cp.tile([1, C], BF16, name="blin")
            nc.sync.dma_start(out=blin_sb[:], in_=blin[:])
            ones_sb = cp.tile([1, 128], BF16, name="ones")
            nc.sync.dma_start(out=ones_sb[:], in_=ones[:])
            id_sb = cp.tile([128, 128], BF16, name="ident")
            nc.sync.dma_start(out=id_sb[:], in_=ident[:])
            r_sb = cp.tile([128, 128], F32, name="rmat")
            nc.sync.dma_start(out=r_sb[:], in_=rmat[:])
            logit_sb = {
                "f": cp.tile([128, ST * C], F32, name="logitf"),
                "b": cp.tile([128, ST * C], F32, name="logitb"),
            }

            # ---- recurrence ----
            lstm_psum = tc.tile_pool(name="psg", bufs=4, space="PSUM")
            pg = lstm_psum.__enter__()
            lstm_psum2 = tc.tile_pool(name="pst", bufs=2, space="PSUM")
            pt = lstm_psum2.__enter__()
            lstm_psum3 = tc.tile_pool(name="psl", bufs=2, space="PSUM")
            pl = lstm_psum3.__enter__()
            cprev, hTprev, gates = {}, {}, {}
            for d in ("f", "b"):
                cprev[d] = sp.tile([128, H], BF16, name=f"c{d}_init", tag=f"c{d}")
                nc.vector.memset(cprev[d][:], 0.0)
                hTprev[d] = sp.tile([128, H], BF16, name=f"hT{d}_init", tag=f"hT{d}")
                nc.vector.memset(hTprev[d][:], 0.0)

            FUNCS = [AF.Sigmoid, AF.Sigmoid, AF.Tanh, AF.Sigmoid]

            def emit_quarters(d, t):
                g4 = []
                for q in range(4):
                    ps = pg.tile([128, 512], F32, name=f"ps{d}{t}{q}", tag="gq")
                    nc.tensor.matmul(ps[:], lhsT=ones_sb[:],
                                     rhs=bias_sb[d][:, 512 * q:512 * (q + 1)],
                                     start=True, stop=False)
                    for k in range(4):
                        lhsT = xt_sb[d][:, k * XW + t: k * XW + t + ST * (NL - 1) + 1: ST]
                        nc.tensor.matmul(ps[:], lhsT=lhsT,
                                         rhs=w_sb[d][:, k * G + 512 * q: k * G + 512 * (q + 1)],
                                         start=False, stop=False)
                    for k in range(4):
                        nc.tensor.matmul(ps[:], lhsT=hTprev[d][:, 128 * k:128 * (k + 1)],
                                         rhs=w_sb[d][:, (4 + k) * G + 512 * q: (4 + k) * G + 512 * (q + 1)],
                                         start=False, stop=(k == 3))
                    gt = sp.tile([128, 512], BF16, name=f"g{d}{t}{q}", tag=f"g{q}{d}")
                    nc.scalar.activation(gt[:], ps[:], FUNCS[q])
                    g4.append(gt)
                gates[d] = g4

            def emit_tail(d, t):
                gi, gf, gg, go = gates[d]
                ig = sp.tile([128, H], BF16, name=f"ig{d}{t}", tag=f"ig{d}")
                nc.vector.tensor_mul(ig[:], gi[:], gg[:])
                fc = sp.tile([128, H], BF16, name=f"fc{d}{t}", tag=f"fc{d}")
                nc.vector.tensor_mul(fc[:], gf[:], cprev[d][:])
                cn = sp.tile([128, H], BF16, name=f"cn{d}{t}", tag=f"cn{d}")
                nc.vector.tensor_add(cn[:], ig[:], fc[:])
                cm = sp.tile([128, H], BF16, name=f"cm{d}{t}", tag=f"c{d}")
                nc.vector.tensor_scalar_mul(cm[:], cn[:], msk_sb[d][:, t:t + 1])
                th = sp.tile([128, H], BF16, name=f"th{d}{t}", tag=f"th{d}")
                nc.scalar.activation(th[:], cm[:], AF.Tanh)
                hn = sp.tile([128, H], BF16, name=f"hn{d}{t}", tag=f"hn{d}")
                nc.vector.tensor_mul(hn[:], go[:], th[:])
                ps = pt.tile([128, H], BF16, name=f"ptr{d}{t}", tag="tr")
                for k in range(4):
                    nc.tensor.transpose(ps[:, 128 * k:128 * (k + 1)],
                                        hn[:, 128 * k:128 * (k + 1)], id_sb[:])
                hT = sp.tile([128, H], BF16, name=f"hT{d}{t}", tag=f"hT{d}")
                nc.vector.tensor_copy(hT[:], ps[:])
                cprev[d], hTprev[d] = cm, hT
                if t >= K:
                    s = t - K
                    psl = pl.tile([128, C], F32, name=f"pl{d}{t}", tag="lg")
                    for k in range(4):
                        nc.tensor.matmul(psl[:], lhsT=hT[:, 128 * k:128 * (k + 1)],
                                         rhs=wlin_sb[d][:, C * k:C * (k + 1)],
                                         start=(k == 0),
                                         stop=(k == 3 and d == "b"))
                    if d == "f":
                        nc.tensor.matmul(psl[:], lhsT=ones_sb[:], rhs=blin_sb[:],
                                         start=False, stop=True)
                    slot = s if d == "f" else (ST - 1 - s)
                    nc.scalar.activation(logit_sb[d][:, C * slot:C * (slot + 1)],
                                         psl[:], AF.Copy)

            for t in range(STEPS):
                emit_quarters("f", t)
                if t > 0:
                    emit_tail("b", t - 1)
                emit_quarters("b", t)
                emit_tail("f", t)
            emit_tail("b", STEPS - 1)

            # fwd logits straight to DRAM rows 17l+s
            nc.sync.dma_start(
                out=logf_d[0:WINW, :].rearrange("(l s) c -> l (s c)", s=ST),
                in_=logit_sb["f"][:],
            )
            # zero the never-written scratch tails so the CRF u-loads read 0
            zt = sp.tile([128, C], F32, name="ztail", tag="ztail")
            nc.vector.memset(zt[:], 0.0)
            nc.sync.dma_start(out=logf_d[WINW:LOGR, :], in_=zt[:])
            nc.sync.dma_start(out=logb_d[WINW:LOGR, :], in_=zt[:])
            lstm_psum3.__exit__(None, None, None)
            lstm_psum2.__exit__(None, None, None)
            lstm_psum.__exit__(None, None, None)

            # ---- reverse bwd logits lanes (R @ logitB), then to DRAM ----
            with tc.tile_pool(name="prev", bufs=1, space="PSUM") as pr:
                psr = pr.tile([128, ST * C], F32, name="psrev")
                nc.tensor.matmul(psr[:, 0:512], lhsT=r_sb[:], rhs=logit_sb["b"][:, 0:512],
                                 start=True, stop=True)
                nc.tensor.matmul(psr[:, 512:ST * C], lhsT=r_sb[:],
                                 rhs=logit_sb["b"][:, 512:ST * C], start=True, stop=True)
                lrev = sp.tile([128, ST * C], F32, name="lrev", tag="lrev")
                nc.vector.tensor_copy(lrev[:], psr[:])
                nc.sync.dma_start(
                    out=logb_d[0:WINW, :].rearrange("(l s) c -> l (s c)", s=ST),
                    in_=lrev[:],
                )

            # ---- CRF ----
            with (
                tc.tile_pool(name="crf", bufs=2) as fp,
                tc.tile_pool(name="crfc", bufs=1) as fc1,
                tc.tile_pool(name="psc", bufs=2, space="PSUM") as pc,
            ):
                shi_sb = fc1.tile([128, 128], BF16, name="shi")
                nc.sync.dma_start(out=shi_sb[:], in_=shi[:])
                slo_sb = fc1.tile([128, 128], BF16, name="slo")
                nc.sync.dma_start(out=slo_sb[:], in_=slo[:])
                valid_sb = fc1.tile([128, NT], F32, name="valid")
                nc.sync.dma_start(out=valid_sb[:], in_=valid[:])

                uf = fc1.tile([128, CRFW], F32, name="uf")
                nc.sync.dma_start(out=uf[:].rearrange("p (T c) -> p T c", c=C),
                                  in_=bass.AP(logf_d[:].tensor, 0,
                                              [[C, 128], [CST * C, NT], [1, C]]))
                ub = fc1.tile([128, CRFW], F32, name="ub")
                nc.sync.dma_start(out=ub[:].rearrange("p (T c) -> p T c", c=C),
                                  in_=bass.AP(logb_d[:].tensor, 64 * C,
                                              [[C, 128], [CST * C, NT], [1, C]]))
                u = fc1.tile([128, CRFW], F32, name="u")
                nc.vector.tensor_add(u[:], uf[:], ub[:])

                xcur = u
                for it in range(NIT + 1):
                    last = it == NIT
                    e = fp.tile([128, CRFW], F32, name=f"e{it}", tag="e")
                    nc.scalar.activation(e[:], xcur[:], AF.Exp)
                    ssum = fp.tile([128, NT], F32, name=f"ss{it}", tag="ss")
                    nc.vector.reduce_sum(ssum[:], e[:].rearrange("p (T c) -> p T c", c=C),
                                         axis=mybir.AxisListType.X)
                    rv = fp.tile([128, NT], F32, name=f"rv{it}", tag="rv")
                    nc.vector.reciprocal(rv[:], ssum[:])
                    if not last:
                        rvv = fp.tile([128, NT], F32, name=f"rvv{it}", tag="rvv")
                        nc.vector.tensor_mul(rvv[:], rv[:], valid_sb[:])
                        p = fp.tile([128, CRFW], BF16, name=f"p{it}", tag="p")
                        nc.vector.tensor_mul(
                            p[:].rearrange("p (T c) -> p T c", c=C),
                            e[:].rearrange("p (T c) -> p T c", c=C),
                            rvv[:].unsqueeze(2).broadcast_to([128, NT, C]))
                        psc = pc.tile([128, CRFW], F32, name=f"pc{it}", tag="pc")
                        for T in range(NT):
                            nc.tensor.matmul(psc[:, C * T:C * (T + 1)], lhsT=shi_sb[:],
                                             rhs=p[:, C * T:C * (T + 1)],
                                             start=True, stop=False)
                            nc.tensor.matmul(psc[:, C * T:C * (T + 1)], lhsT=slo_sb[:],
                                             rhs=p[:, C * T:C * (T + 1)],
                                             start=False, stop=True)
                        xn = fp.tile([128, CRFW], F32, name=f"x{it}", tag="x")
                        nc.vector.tensor_add(xn[:], u[:], psc[:])
                        xcur = xn
                    else:
                        pout = fp.tile([128, CRFW], F32, name="pout", tag="p")
                        nc.vector.tensor_mul(
                            pout[:].rearrange("p (T c) -> p T c", c=C),
                            e[:].rearrange("p (T c) -> p T c", c=C),
                            rv[:].unsqueeze(2).broadcast_to([128, NT, C]))
                        nc.sync.dma_start(
                            out=bass.AP(out[:].tensor, 25 * C, [[C, CST], [CST * C, NT], [1, C]]),
                            in_=pout[25:25 + CST, :].rearrange("p (T c) -> p T c", c=C))
                        nc.sync.dma_start(
                            out=bass.AP(out[:].tensor, 0, [[C, 25], [1, C]]),
                            in_=pout[0:25, 0:C])

    nc.compile()
    return nc


def _prep(inputs):
    I = {k: np.asarray(v, np.float32) for k, v in inputs.items()}
    x = I["batch"]
    xr = x[::-1]
    bf = ml_dtypes.bfloat16

    Wf = np.concatenate([I["W_ih_f"].T, I["W_hh_f"].T], 0)  # (1024, 2048)
    Wb = np.concatenate([I["W_ih_b"].T, I["W_hh_b"].T], 0)
    biasf = (I["b_ih_f"] + I["b_hh_f"])[None, :]
    biasb = (I["b_ih_b"] + I["b_hh_b"])[None, :]
    WlinT = I["W_lin"].T  # (1024, 32)

    half = FILT // 2
    dd = np.arange(-half, half + 1, dtype=np.float32)
    kern = np.exp(-(dd * I["inv_smoothness_theta"][0]) ** 2 / 2)
    kern[half] = 0.0
    kern *= I["smoothness_weight"]
    S = np.zeros((128, 128), np.float32)
    for i in range(128):
        for j in range(max(0, i - half), min(128, i + half + 1)):
            if i != j:
                S[i, j] = kern[j - i + half]
    S_hi = S.astype(bf).astype(np.float32)
    S_lo = (S - S_hi).astype(bf)

    shared = dict(
        wf=Wf.reshape(8, 128, G).astype(bf),
        wb=Wb.reshape(8, 128, G).astype(bf),
        biasf=biasf.astype(bf), biasb=biasb.astype(bf),
        wlinf=WlinT[:512].reshape(4, 128, C).astype(bf),
        wlinb=WlinT[512:].reshape(4, 128, C).astype(bf),
        blin=I["b_lin"][None, :].astype(bf),
        ones=np.ones((1, 128), bf),
        ident=np.eye(128, dtype=np.float32).astype(bf),
        rmat=np.eye(128, dtype=np.float32)[::-1].copy(),
        shi=S_hi.astype(bf), slo=S_lo,
    )

    def window(src, W0):
        w = np.zeros((K + WINW, EMB), np.float32)
        lo, hi = W0 - K, W0 + WINW
        slo, shi_ = max(lo, 0), min(hi, SEQ)
        if shi_ > slo:
            w[slo - lo:shi_ - lo] = src[slo:shi_]
        return np.ascontiguousarray(w.T).reshape(4, 128, K + WINW).astype(bf)

    st = np.arange(STEPS)[None, :]
    ll = np.arange(NL)[:, None] * ST
    pp = np.arange(128)[:, None]
    TT = np.arange(NT)[None, :] * CST
    in_maps = []
    for c in range(NCORES):
        Wc = 2048 * c - 32
        Wr = 2048 * (7 - c) - 32
        gpos = Wc + TT + pp
        m = dict(shared)
        m["xtf"] = window(x, Wc)
        m["xtb"] = window(xr, Wr)
        m["mf"] = ((ll + st + Wc - K) >= 0).astype(np.float32)
        m["mb"] = ((ll + st + Wr - K) >= 0).astype(np.float32)
        m["valid"] = ((gpos >= 0) & (gpos < SEQ) & (TT + pp < WINW)).astype(np.float32)
        in_maps.append(m)
    return in_maps


def _run(inputs, trace=False, trace_cores=None):
    if "nc" not in _CACHE:
        _CACHE["nc"] = _build()
    nc = _CACHE["nc"]
    in_maps = _prep(inputs)
    kw = {}
    if trace:
        import types
        try:
            import trn_agent_boot.trn_boot as tb
            hook = tb._ntff_profile_via_ctypes("/opt/axon/libaxon_pjrt.so")
            mod = types.ModuleType("antenv.axon_hooks")
            mod.get_axon_ntff_profile_hook = lambda: hook
            sys.modules.setdefault("antenv.axon_hooks", mod)
        except Exception:
            pass
        kw = dict(trace=True, trace_cores=trace_cores or list(range(NCORES)))
    res = run_bass_kernel_spmd(nc, in_maps, list(range(NCORES)), **kw)
    full = np.zeros((SEQ, C), np.float32)
    for c in range(NCORES):
        full[2048 * c:2048 * (c + 1)] = res.results[c]["out"][32:2080]
    return full, res


def kernel(**inputs):
    full, _ = _run(inputs)
    return full



# revision 2
# speedup vs baseline: 1.6419x; 1.6419x over previous
"""BiLSTM + mean-field CRF on 8 Trainium2 NeuronCores.

Strategy: the single 16384-long sequence is split into 8 contiguous
2048-position core slices (data-parallel across cores). Inside each core the
sequence is further split into 128 lanes of 17 consecutive positions each;
every lane warm-starts K=20 steps early from zero state (LSTM forget gates
~sigmoid(small pre-activations) => state influence decays ~0.5^K, so the
truncation error is far below bf16 noise). Both LSTM directions run as
batched 128-lane recurrences; gates come from one fused PSUM accumulation
[x_t; h_{t-1}] @ [W_ih^T; W_hh^T] + bias (bias via a K=1 matmul). The hidden
state is re-transposed each step with TensorE transpose so it can serve as
the stationary operand of the next step's matmul. Logits are computed per
step with tiny N=32 matmuls from the already-transposed state, scattered to
DRAM in position order, and the CRF (conv kernel as a banded 128x128
Toeplitz matmul, softmax via free-dim reduce in a position-on-partitions
layout) runs on position tiles of 128 at stride 78 whose edges erode 5
positions per iteration.
"""
import sys

sys.path.insert(0, "/opt/trn_rl_repo")

import numpy as np
import ml_dtypes

import concourse.bass as bass
import concourse.bacc as bacc
import concourse.mybir as mybir
from concourse.tile import TileContext
from concourse.bass_utils import run_bass_kernel_spmd

F32 = mybir.dt.float32
BF16 = mybir.dt.bfloat16
AF = mybir.ActivationFunctionType

SEQ, EMB, H, G, C = 16384, 512, 512, 2048, 32
NCORES = 8
K = 8                  # halo warm-up steps
ST = 17                # positions per lane
NL = 128               # lanes
STEPS = K + ST         # 37
WINW = NL * ST         # 2176
XW = K + WINW          # 2196 x-window columns per k-tile
CST, NT = 78, 28       # CRF tile stride / count
CRFW = NT * C          # 896
LOGR = 2304            # logits scratch rows (>= 78*27+64+128)
OUTR = 2240            # output rows per core
FILT, NIT = 11, 5

_CACHE = {}


def _build():
    nc = bacc.Bacc("TRN2", target_bir_lowering=False, debug=False, num_devices=NCORES)

    def din(name, shape, dt=BF16):
        return nc.dram_tensor(name, shape, dt, kind="ExternalInput")

    xtf = din("xtf", [4, 128, XW])
    xtb = din("xtb", [4, 128, XW])
    wf = din("wf", [8, 128, G])
    wb = din("wb", [8, 128, G])
    biasf = din("biasf", [1, G])
    biasb = din("biasb", [1, G])
    wlinf = din("wlinf", [4, 128, C])
    wlinb = din("wlinb", [4, 128, C])
    blin = din("blin", [1, C])
    ones = din("ones", [1, 128])
    ident = din("ident", [128, 128])
    rmat = din("rmat", [128, 128], F32)
    shi = din("shi", [128, 128])
    slo = din("slo", [128, 128])
    mf = din("mf", [128, STEPS], F32)
    mb = din("mb", [128, STEPS], F32)
    valid = din("valid", [128, NT], F32)

    out = nc.dram_tensor("out", [OUTR, C], F32, kind="ExternalOutput")
    logf_d = nc.dram_tensor("logf_d", [LOGR, C], F32)
    logb_d = nc.dram_tensor("logb_d", [LOGR, C], F32)

    with TileContext(nc) as tc:
        with (
            tc.tile_pool(name="consts", bufs=1) as cp,
            tc.tile_pool(name="state", bufs=2) as sp,
        ):
            # ---- load constants/inputs into SBUF ----
            xt_sb, w_sb, bias_sb, wlin_sb, msk_sb = {}, {}, {}, {}, {}
            for d, (xs, ws, bs, wl, ms) in (
                ("f", (xtf, wf, biasf, wlinf, mf)),
                ("b", (xtb, wb, biasb, wlinb, mb)),
            ):
                t = cp.tile([128, 4 * XW], BF16, name=f"xt{d}")
                for k in range(4):
                    nc.sync.dma_start(out=t[:, k * XW:(k + 1) * XW], in_=xs[k])
                xt_sb[d] = t
                t = cp.tile([128, 8 * G], BF16, name=f"w{d}")
                for k in range(8):
                    nc.sync.dma_start(out=t[:, k * G:(k + 1) * G], in_=ws[k])
                w_sb[d] = t
                t = cp.tile([1, G], BF16, name=f"bias{d}")
                nc.sync.dma_start(out=t[:], in_=bs[:])
                bias_sb[d] = t
                t = cp.tile([128, 4 * C], BF16, name=f"wlin{d}")
                for k in range(4):
                    nc.sync.dma_start(out=t[:, k * C:(k + 1) * C], in_=wl[k])
                wlin_sb[d] = t
                t = cp.tile([128, STEPS], F32, name=f"msk{d}")
                nc.sync.dma_start(out=t[:], in_=ms[:])
                msk_sb[d] = t

            blin_sb = cp.tile([1, C], BF16, name="blin")
            nc.sync.dma_start(out=blin_sb[:], in_=blin[:])
            ones_sb = cp.tile([1, 128], BF16, name="ones")
            nc.sync.dma_start(out=ones_sb[:], in_=ones[:])
            id_sb = cp.tile([128, 128], BF16, name="ident")
            nc.sync.dma_start(out=id_sb[:], in_=ident[:])
            r_sb = cp.tile([128, 128], F32, name="rmat")
            nc.sync.dma_start(out=r_sb[:], in_=rmat[:])
            logit_sb = {
                "f": cp.tile([128, ST * C], F32, name="logitf"),
                "b": cp.tile([128, ST * C], F32, name="logitb"),
            }

            # ---- recurrence ----
            lstm_psum = tc.tile_pool(name="psg", bufs=4, space="PSUM")
            pg = lstm_psum.__enter__()
            lstm_psum2 = tc.tile_pool(name="pst", bufs=2, space="PSUM")
            pt = lstm_psum2.__enter__()
            lstm_psum3 = tc.tile_pool(name="psl", bufs=2, space="PSUM")
            pl = lstm_psum3.__enter__()
            cprev, hTprev, gates = {}, {}, {}
            for d in ("f", "b"):
                cprev[d] = sp.tile([128, H], BF16, name=f"c{d}_init", tag=f"c{d}")
                nc.vector.memset(cprev[d][:], 0.0)
                hTprev[d] = sp.tile([128, H], BF16, name=f"hT{d}_init", tag=f"hT{d}")
                nc.vector.memset(hTprev[d][:], 0.0)

            FUNCS = [AF.Sigmoid, AF.Sigmoid, AF.Tanh, AF.Sigmoid]

            def emit_quarters(d, t):
                g4 = []
                for q in range(4):
                    ps = pg.tile([128, 512], F32, name=f"ps{d}{t}{q}", tag="gq")
                    nc.tensor.matmul(ps[:], lhsT=ones_sb[:],
                                     rhs=bias_sb[d][:, 512 * q:512 * (q + 1)],
                                     start=True, stop=False)
                    for k in range(4):
                        lhsT = xt_sb[d][:, k * XW + t: k * XW + t + ST * (NL - 1) + 1: ST]
                        nc.tensor.matmul(ps[:], lhsT=lhsT,
                                         rhs=w_sb[d][:, k * G + 512 * q: k * G + 512 * (q + 1)],
                                         start=False, stop=False)
                    for k in range(4):
                        nc.tensor.matmul(ps[:], lhsT=hTprev[d][:, 128 * k:128 * (k + 1)],
                                         rhs=w_sb[d][:, (4 + k) * G + 512 * q: (4 + k) * G + 512 * (q + 1)],
                                         start=False, stop=(k == 3))
                    gt = sp.tile([128, 512], BF16, name=f"g{d}{t}{q}", tag=f"g{q}{d}")
                    nc.scalar.activation(gt[:], ps[:], FUNCS[q])
                    g4.append(gt)
                gates[d] = g4

            def emit_tail(d, t):
                gi, gf, gg, go = gates[d]
                ig = sp.tile([128, H], BF16, name=f"ig{d}{t}", tag=f"ig{d}")
                nc.vector.tensor_mul(ig[:], gi[:], gg[:])
                fc = sp.tile([128, H], BF16, name=f"fc{d}{t}", tag=f"fc{d}")
                nc.vector.tensor_mul(fc[:], gf[:], cprev[d][:])
                cn = sp.tile([128, H], BF16, name=f"cn{d}{t}", tag=f"cn{d}")
                nc.vector.tensor_add(cn[:], ig[:], fc[:])
                cm = sp.tile([128, H], BF16, name=f"cm{d}{t}", tag=f"c{d}")
                nc.vector.tensor_scalar_mul(cm[:], cn[:], msk_sb[d][:, t:t + 1])
                th = sp.tile([128, H], BF16, name=f"th{d}{t}", tag=f"th{d}")
                nc.scalar.activation(th[:], cm[:], AF.Tanh)
                hn = sp.tile([128, H], BF16, name=f"hn{d}{t}", tag=f"hn{d}")
                nc.vector.tensor_mul(hn[:], go[:], th[:])
                ps = pt.tile([128, H], BF16, name=f"ptr{d}{t}", tag="tr")
                for k in range(4):
                    nc.tensor.transpose(ps[:, 128 * k:128 * (k + 1)],
                                        hn[:, 128 * k:128 * (k + 1)], id_sb[:])
                hT = sp.tile([128, H], BF16, name=f"hT{d}{t}", tag=f"hT{d}")
                nc.vector.tensor_copy(hT[:], ps[:])
                cprev[d], hTprev[d] = cm, hT
                if t >= K:
                    s = t - K
                    psl = pl.tile([128, C], F32, name=f"pl{d}{t}", tag="lg")
                    for k in range(4):
                        nc.tensor.matmul(psl[:], lhsT=hT[:, 128 * k:128 * (k + 1)],
                                         rhs=wlin_sb[d][:, C * k:C * (k + 1)],
                                         start=(k == 0),
                                         stop=(k == 3 and d == "b"))
                    if d == "f":
                        nc.tensor.matmul(psl[:], lhsT=ones_sb[:], rhs=blin_sb[:],
                                         start=False, stop=True)
                    slot = s if d == "f" else (ST - 1 - s)
                    nc.scalar.activation(logit_sb[d][:, C * slot:C * (slot + 1)],
                                         psl[:], AF.Copy)

            for t in range(STEPS):
                emit_quarters("f", t)
                if t > 0:
                    emit_tail("b", t - 1)
                emit_quarters("b", t)
                emit_tail("f", t)
            emit_tail("b", STEPS - 1)

            # fwd logits straight to DRAM rows 17l+s
            nc.sync.dma_start(
                out=logf_d[0:WINW, :].rearrange("(l s) c -> l (s c)", s=ST),
                in_=logit_sb["f"][:],
            )
            # zero the never-written scratch tails so the CRF u-loads read 0
            zt = sp.tile([128, C], F32, name="ztail", tag="ztail")
            nc.vector.memset(zt[:], 0.0)
            nc.sync.dma_start(out=logf_d[WINW:LOGR, :], in_=zt[:])
            nc.sync.dma_start(out=logb_d[WINW:LOGR, :], in_=zt[:])
            lstm_psum3.__exit__(None, None, None)
            lstm_psum2.__exit__(None, None, None)
            lstm_psum.__exit__(None, None, None)

            # ---- reverse bwd logits lanes (R @ logitB), then to DRAM ----
            with tc.tile_pool(name="prev", bufs=1, space="PSUM") as pr:
                psr = pr.tile([128, ST * C], F32, name="psrev")
                nc.tensor.matmul(psr[:, 0:512], lhsT=r_sb[:], rhs=logit_sb["b"][:, 0:512],
                                 start=True, stop=True)
                nc.tensor.matmul(psr[:, 512:ST * C], lhsT=r_sb[:],
                                 rhs=logit_sb["b"][:, 512:ST * C], start=True, stop=True)
                lrev = sp.tile([128, ST * C], F32, name="lrev", tag="lrev")
                nc.vector.tensor_copy(lrev[:], psr[:])
                nc.sync.dma_start(
                    out=logb_d[0:WINW, :].rearrange("(l s) c -> l (s c)", s=ST),
                    in_=lrev[:],
                )

            # ---- CRF ----
            with (
                tc.tile_pool(name="crf", bufs=2) as fp,
                tc.tile_pool(name="crfc", bufs=1) as fc1,
                tc.tile_pool(name="psc", bufs=2, space="PSUM") as pc,
            ):
                shi_sb = fc1.tile([128, 128], BF16, name="shi")
                nc.sync.dma_start(out=shi_sb[:], in_=shi[:])
                slo_sb = fc1.tile([128, 128], BF16, name="slo")
                nc.sync.dma_start(out=slo_sb[:], in_=slo[:])
                valid_sb = fc1.tile([128, NT], F32, name="valid")
                nc.sync.dma_start(out=valid_sb[:], in_=valid[:])

                uf = fc1.tile([128, CRFW], F32, name="uf")
                nc.sync.dma_start(out=uf[:].rearrange("p (T c) -> p T c", c=C),
                                  in_=bass.AP(logf_d[:].tensor, 0,
                                              [[C, 128], [CST * C, NT], [1, C]]))
                ub = fc1.tile([128, CRFW], F32, name="ub")
                nc.sync.dma_start(out=ub[:].rearrange("p (T c) -> p T c", c=C),
                                  in_=bass.AP(logb_d[:].tensor, 64 * C,
                                              [[C, 128], [CST * C, NT], [1, C]]))
                u = fc1.tile([128, CRFW], F32, name="u")
                nc.vector.tensor_add(u[:], uf[:], ub[:])

                xcur = u
                for it in range(NIT + 1):
                    last = it == NIT
                    e = fp.tile([128, CRFW], F32, name=f"e{it}", tag="e")
                    nc.scalar.activation(e[:], xcur[:], AF.Exp)
                    ssum = fp.tile([128, NT], F32, name=f"ss{it}", tag="ss")
                    nc.vector.reduce_sum(ssum[:], e[:].rearrange("p (T c) -> p T c", c=C),
                                         axis=mybir.AxisListType.X)
                    rv = fp.tile([128, NT], F32, name=f"rv{it}", tag="rv")
                    nc.vector.reciprocal(rv[:], ssum[:])
                    if not last:
                        rvv = fp.tile([128, NT], F32, name=f"rvv{it}", tag="rvv")
                        nc.vector.tensor_mul(rvv[:], rv[:], valid_sb[:])
                        p = fp.tile([128, CRFW], BF16, name=f"p{it}", tag="p")
                        nc.vector.tensor_mul(
                            p[:].rearrange("p (T c) -> p T c", c=C),
                            e[:].rearrange("p (T c) -> p T c", c=C),
                            rvv[:].unsqueeze(2).broadcast_to([128, NT, C]))
                        psc = pc.tile([128, CRFW], F32, name=f"pc{it}", tag="pc")
                        for T in range(NT):
                            nc.tensor.matmul(psc[:, C * T:C * (T + 1)], lhsT=shi_sb[:],
                                             rhs=p[:, C * T:C * (T + 1)],
                                             start=True, stop=False)
                            nc.tensor.matmul(psc[:, C * T:C * (T + 1)], lhsT=slo_sb[:],
                                             rhs=p[:, C * T:C * (T + 1)],
                                             start=False, stop=True)
                        xn = fp.tile([128, CRFW], F32, name=f"x{it}", tag="x")
                        nc.vector.tensor_add(xn[:], u[:], psc[:])
                        xcur = xn
                    else:
                        pout = fp.tile([128, CRFW], F32, name="pout", tag="p")
                        nc.vector.tensor_mul(
                            pout[:].rearrange("p (T c) -> p T c", c=C),
                            e[:].rearrange("p (T c) -> p T c", c=C),
                            rv[:].unsqueeze(2).broadcast_to([128, NT, C]))
                        nc.sync.dma_start(
                            out=bass.AP(out[:].tensor, 25 * C, [[C, CST], [CST * C, NT], [1, C]]),
                            in_=pout[25:25 + CST, :].rearrange("p (T c) -> p T c", c=C))
                        nc.sync.dma_start(
                            out=bass.AP(out[:].tensor, 0, [[C, 25], [1, C]]),
                            in_=pout[0:25, 0:C])

    nc.compile()
    return nc


def _prep(inputs):
    I = {k: np.asarray(v, np.float32) for k, v in inputs.items()}
    x = I["batch"]
    xr = x[::-1]
    bf = ml_dtypes.bfloat16

    Wf = np.concatenate([I["W_ih_f"].T, I["W_hh_f"].T], 0)  # (1024, 2048)
    Wb = np.concatenate([I["W_ih_b"].T, I["W_hh_b"].T], 0)
    biasf = (I["b_ih_f"] + I["b_hh_f"])[None, :]
    biasb = (I["b_ih_b"] + I["b_hh_b"])[None, :]
    WlinT = I["W_lin"].T  # (1024, 32)

    half = FILT // 2
    dd = np.arange(-half, half + 1, dtype=np.float32)
    kern = np.exp(-(dd * I["inv_smoothness_theta"][0]) ** 2 / 2)
    kern[half] = 0.0
    kern *= I["smoothness_weight"]
    S = np.zeros((128, 128), np.float32)
    for i in range(128):
        for j in range(max(0, i - half), min(128, i + half + 1)):
            if i != j:
                S[i, j] = kern[j - i + half]
    S_hi = S.astype(bf).astype(np.float32)
    S_lo = (S - S_hi).astype(bf)

    shared = dict(
        wf=Wf.reshape(8, 128, G).astype(bf),
        wb=Wb.reshape(8, 128, G).astype(bf),
        biasf=biasf.astype(bf), biasb=biasb.astype(bf),
        wlinf=WlinT[:512].reshape(4, 128, C).astype(bf),
        wlinb=WlinT[512:].reshape(4, 128, C).astype(bf),
        blin=I["b_lin"][None, :].astype(bf),
        ones=np.ones((1, 128), bf),
        ident=np.eye(128, dtype=np.float32).astype(bf),
        rmat=np.eye(128, dtype=np.float32)[::-1].copy(),
        shi=S_hi.astype(bf), slo=S_lo,
    )

    def window(src, W0):
        w = np.zeros((K + WINW, EMB), np.float32)
        lo, hi = W0 - K, W0 + WINW
        slo, shi_ = max(lo, 0), min(hi, SEQ)
        if shi_ > slo:
            w[slo - lo:shi_ - lo] = src[slo:shi_]
        return np.ascontiguousarray(w.T).reshape(4, 128, K + WINW).astype(bf)

    st = np.arange(STEPS)[None, :]
    ll = np.arange(NL)[:, None] * ST
    pp = np.arange(128)[:, None]
    TT = np.arange(NT)[None, :] * CST
    in_maps = []
    for c in range(NCORES):
        Wc = 2048 * c - 32
        Wr = 2048 * (7 - c) - 32
        gpos = Wc + TT + pp
        m = dict(shared)
        m["xtf"] = window(x, Wc)
        m["xtb"] = window(xr, Wr)
        m["mf"] = ((ll + st + Wc - K) >= 0).astype(np.float32)
        m["mb"] = ((ll + st + Wr - K) >= 0).astype(np.float32)
        m["valid"] = ((gpos >= 0) & (gpos < SEQ) & (TT + pp < WINW)).astype(np.float32)
        in_maps.append(m)
    return in_maps


def _run(inputs, trace=False, trace_cores=None):
    if "nc" not in _CACHE:
        _CACHE["nc"] = _build()
    nc = _CACHE["nc"]
    in_maps = _prep(inputs)
    kw = {}
    if trace:
        import types
        try:
            import trn_agent_boot.trn_boot as tb
            hook = tb._ntff_profile_via_ctypes("/opt/axon/libaxon_pjrt.so")
            mod = types.ModuleType("antenv.axon_hooks")
            mod.get_axon_ntff_profile_hook = lambda: hook
            sys.modules.setdefault("antenv.axon_hooks", mod)
        except Exception:
            pass
        kw = dict(trace=True, trace_cores=trace_cores or list(range(NCORES)))
    res = run_bass_kernel_spmd(nc, in_maps, list(range(NCORES)), **kw)
    full = np.zeros((SEQ, C), np.float32)
    for c in range(NCORES):
        full[2048 * c:2048 * (c + 1)] = res.results[c]["out"][32:2080]
    return full, res


def kernel(**inputs):
    full, _ = _run(inputs)
    return full



# revision 11
# speedup vs baseline: 1.9862x; 1.2097x over previous
"""BiLSTM + mean-field CRF on 8 Trainium2 NeuronCores.

Strategy: the single 16384-long sequence is split into 8 contiguous
2048-position core slices (data-parallel across cores). Inside each core the
sequence is further split into 128 lanes of 17 consecutive positions each;
every lane warm-starts K=20 steps early from zero state (LSTM forget gates
~sigmoid(small pre-activations) => state influence decays ~0.5^K, so the
truncation error is far below bf16 noise). Both LSTM directions run as
batched 128-lane recurrences; gates come from one fused PSUM accumulation
[x_t; h_{t-1}] @ [W_ih^T; W_hh^T] + bias (bias via a K=1 matmul). The hidden
state is re-transposed each step with TensorE transpose so it can serve as
the stationary operand of the next step's matmul. Logits are computed per
step with tiny N=32 matmuls from the already-transposed state, scattered to
DRAM in position order, and the CRF (conv kernel as a banded 128x128
Toeplitz matmul, softmax via free-dim reduce in a position-on-partitions
layout) runs on position tiles of 128 at stride 78 whose edges erode 5
positions per iteration.
"""
import sys

sys.path.insert(0, "/opt/trn_rl_repo")

import numpy as np
import ml_dtypes

import concourse.bass as bass
import concourse.bacc as bacc
import concourse.mybir as mybir
from concourse.tile import TileContext
from concourse.bass_utils import run_bass_kernel_spmd

F32 = mybir.dt.float32
BF16 = mybir.dt.bfloat16
AF = mybir.ActivationFunctionType

SEQ, EMB, H, G, C = 16384, 512, 512, 2048, 32
NCORES = 8
K = 8                  # halo warm-up steps
ST = 17                # positions per lane
NL = 128               # lanes
STEPS = K + ST         # 37
WINW = NL * ST         # 2176
XW = K + WINW          # 2196 x-window columns per k-tile
CST, NT = 78, 28       # CRF tile stride / count
CRFW = NT * C          # 896
LOGR = 2304            # logits scratch rows (>= 78*27+64+128)
OUTR = 2240            # output rows per core
FILT, NIT = 11, 5

_CACHE = {}


def _build():
    nc = bacc.Bacc("TRN2", target_bir_lowering=False, debug=False, num_devices=NCORES)

    def din(name, shape, dt=BF16):
        return nc.dram_tensor(name, shape, dt, kind="ExternalInput")

    xtf = din("xtf", [4, 128, XW])
    xtb = din("xtb", [4, 128, XW])
    wf = din("wf", [8, 128, G])
    wb = din("wb", [8, 128, G])
    biasf = din("biasf", [128, G])
    biasb = din("biasb", [128, G])
    wlinf = din("wlinf", [4, 128, C])
    wlinb = din("wlinb", [4, 128, C])
    blin = din("blin", [1, C])
    ones = din("ones", [1, 128])
    ident = din("ident", [128, 128])
    rmat = din("rmat", [128, 128], F32)
    shi = din("shi", [128, 128])
    slo = din("slo", [128, 128])
    mf = din("mf", [128, STEPS], F32)
    mb = din("mb", [128, STEPS], F32)
    valid = din("valid", [128, NT], F32)

    out = nc.dram_tensor("out", [128, CRFW], F32, kind="ExternalOutput")
    logf_d = nc.dram_tensor("logf_d", [LOGR, C], F32)
    logb_d = nc.dram_tensor("logb_d", [LOGR, C], F32)

    with TileContext(nc) as tc:
        with (
            tc.tile_pool(name="consts", bufs=1) as cp,
            tc.tile_pool(name="state", bufs=2) as sp,
        ):
            # ---- load constants/inputs into SBUF ----
            xt_sb, w_sb, bias_sb, wlin_sb, msk_sb = {}, {}, {}, {}, {}
            for d, (xs, ws, bs, wl, ms) in (
                ("f", (xtf, wf, biasf, wlinf, mf)),
                ("b", (xtb, wb, biasb, wlinb, mb)),
            ):
                t = cp.tile([128, 4 * XW], BF16, name=f"xt{d}")
                for k in range(4):
                    nc.sync.dma_start(out=t[:, k * XW:(k + 1) * XW], in_=xs[k])
                xt_sb[d] = t
                t = cp.tile([128, 8 * G], BF16, name=f"w{d}")
                for k in range(8):
                    nc.sync.dma_start(out=t[:, k * G:(k + 1) * G], in_=ws[k])
                w_sb[d] = t
                t = cp.tile([128, G], BF16, name=f"bias{d}")
                nc.sync.dma_start(out=t[:], in_=bs[:])
                bias_sb[d] = t
                t = cp.tile([128, 4 * C], BF16, name=f"wlin{d}")
                for k in range(4):
                    nc.sync.dma_start(out=t[:, k * C:(k + 1) * C], in_=wl[k])
                wlin_sb[d] = t
                t = cp.tile([128, STEPS], F32, name=f"msk{d}")
                nc.sync.dma_start(out=t[:], in_=ms[:])
                msk_sb[d] = t

            blin_sb = cp.tile([1, C], BF16, name="blin")
            nc.sync.dma_start(out=blin_sb[:], in_=blin[:])
            ones_sb = cp.tile([1, 128], BF16, name="ones")
            nc.sync.dma_start(out=ones_sb[:], in_=ones[:])
            id_sb = cp.tile([128, 128], BF16, name="ident")
            nc.sync.dma_start(out=id_sb[:], in_=ident[:])
            r_sb = cp.tile([128, 128], F32, name="rmat")
            nc.sync.dma_start(out=r_sb[:], in_=rmat[:])
            logit_sb = {
                "f": cp.tile([128, ST * C], F32, name="logitf"),
                "b": cp.tile([128, ST * C], F32, name="logitb"),
            }

            # ---- recurrence ----
            lstm_psum = tc.tile_pool(name="psg", bufs=4, space="PSUM")
            pg = lstm_psum.__enter__()
            lstm_psum2 = tc.tile_pool(name="pst", bufs=2, space="PSUM")
            pt = lstm_psum2.__enter__()
            lstm_psum3 = tc.tile_pool(name="psl", bufs=2, space="PSUM")
            pl = lstm_psum3.__enter__()
            cprev, hTprev, gates = {}, {}, {}
            for d in ("f", "b"):
                cprev[d] = sp.tile([128, H], BF16, name=f"c{d}_init", tag=f"c{d}")
                nc.vector.memset(cprev[d][:], 0.0)
                hTprev[d] = sp.tile([128, H], BF16, name=f"hT{d}_init", tag=f"hT{d}")
                nc.vector.memset(hTprev[d][:], 0.0)

            FUNCS = [AF.Sigmoid, AF.Sigmoid, AF.Tanh, AF.Sigmoid]

            def emit_quarters(d, t):
                ps4 = [pg.tile([128, 512], F32, name=f"ps{d}{t}{q}", tag="gq")
                       for q in range(4)]
                for k in range(4):
                    lhsT = xt_sb[d][:, k * XW + t: k * XW + t + ST * (NL - 1) + 1: ST]
                    for q in range(4):
                        nc.tensor.matmul(ps4[q][:], lhsT=lhsT,
                                         rhs=w_sb[d][:, k * G + 512 * q: k * G + 512 * (q + 1)],
                                         start=(k == 0), stop=False)
                for k in range(4):
                    lhsT = hTprev[d][:, 128 * k:128 * (k + 1)]
                    for q in range(4):
                        nc.tensor.matmul(ps4[q][:], lhsT=lhsT,
                                         rhs=w_sb[d][:, (4 + k) * G + 512 * q: (4 + k) * G + 512 * (q + 1)],
                                         start=False, stop=(k == 3))
                g4 = []
                for q in range(4):
                    pre = sp.tile([128, 512], BF16, name=f"pre{d}{t}{q}", tag=f"pre{q}{d}")
                    nc.vector.tensor_add(pre[:], ps4[q][:],
                                         bias_sb[d][:, 512 * q:512 * (q + 1)])
                    gt = sp.tile([128, 512], BF16, name=f"g{d}{t}{q}", tag=f"g{q}{d}")
                    nc.scalar.activation(gt[:], pre[:], FUNCS[q])
                    g4.append(gt)
                gates[d] = g4

            def emit_tail(d, t):
                gi, gf, gg, go = gates[d]
                mskt = msk_sb[d][:, t:t + 1]
                ig = sp.tile([128, H], BF16, name=f"ig{d}{t}", tag=f"ig{d}")
                nc.vector.scalar_tensor_tensor(
                    ig[:], gi[:], mskt, gg[:],
                    op0=mybir.AluOpType.mult, op1=mybir.AluOpType.mult)
                fc = sp.tile([128, H], BF16, name=f"fc{d}{t}", tag=f"fc{d}")
                nc.vector.scalar_tensor_tensor(
                    fc[:], gf[:], mskt, cprev[d][:],
                    op0=mybir.AluOpType.mult, op1=mybir.AluOpType.mult)
                cm = sp.tile([128, H], BF16, name=f"cm{d}{t}", tag=f"c{d}")
                nc.vector.tensor_add(cm[:], ig[:], fc[:])
                th = sp.tile([128, H], BF16, name=f"th{d}{t}", tag=f"th{d}")
                nc.scalar.activation(th[:], cm[:], AF.Tanh)
                hn = sp.tile([128, H], BF16, name=f"hn{d}{t}", tag=f"hn{d}")
                nc.vector.tensor_mul(hn[:], go[:], th[:])
                ps = pt.tile([128, H], BF16, name=f"ptr{d}{t}", tag="tr")
                for k in range(4):
                    nc.tensor.transpose(ps[:, 128 * k:128 * (k + 1)],
                                        hn[:, 128 * k:128 * (k + 1)], id_sb[:])
                hT = sp.tile([128, H], BF16, name=f"hT{d}{t}", tag=f"hT{d}")
                nc.scalar.activation(hT[:], ps[:], AF.Copy)
                cprev[d], hTprev[d] = cm, hT
                if t >= K:
                    s = t - K
                    psl = pl.tile([128, C], F32, name=f"pl{d}{t}", tag="lg")
                    for k in range(4):
                        nc.tensor.matmul(psl[:], lhsT=hT[:, 128 * k:128 * (k + 1)],
                                         rhs=wlin_sb[d][:, C * k:C * (k + 1)],
                                         start=(k == 0),
                                         stop=(k == 3 and d == "b"))
                    if d == "f":
                        nc.tensor.matmul(psl[:], lhsT=ones_sb[:], rhs=blin_sb[:],
                                         start=False, stop=True)
                    slot = s if d == "f" else (ST - 1 - s)
                    nc.scalar.activation(logit_sb[d][:, C * slot:C * (slot + 1)],
                                         psl[:], AF.Copy)

            for t in range(STEPS):
                emit_quarters("f", t)
                if t > 0:
                    emit_tail("b", t - 1)
                emit_quarters("b", t)
                emit_tail("f", t)
            emit_tail("b", STEPS - 1)

            # fwd logits straight to DRAM rows 17l+s
            nc.sync.dma_start(
                out=logf_d[0:WINW, :].rearrange("(l s) c -> l (s c)", s=ST),
                in_=logit_sb["f"][:],
            )
            # zero the never-written scratch tails so the CRF u-loads read 0
            zt = sp.tile([128, C], F32, name="ztail", tag="ztail")
            nc.vector.memset(zt[:], 0.0)
            nc.sync.dma_start(out=logf_d[WINW:LOGR, :], in_=zt[:])
            nc.sync.dma_start(out=logb_d[WINW:LOGR, :], in_=zt[:])
            lstm_psum3.__exit__(None, None, None)
            lstm_psum2.__exit__(None, None, None)
            lstm_psum.__exit__(None, None, None)

            # ---- reverse bwd logits lanes (R @ logitB), then to DRAM ----
            with tc.tile_pool(name="prev", bufs=1, space="PSUM") as pr:
                psr = pr.tile([128, ST * C], F32, name="psrev")
                nc.tensor.matmul(psr[:, 0:512], lhsT=r_sb[:], rhs=logit_sb["b"][:, 0:512],
                                 start=True, stop=True)
                nc.tensor.matmul(psr[:, 512:ST * C], lhsT=r_sb[:],
                                 rhs=logit_sb["b"][:, 512:ST * C], start=True, stop=True)
                lrev = sp.tile([128, ST * C], F32, name="lrev", tag="lrev")
                nc.vector.tensor_copy(lrev[:], psr[:])
                nc.sync.dma_start(
                    out=logb_d[0:WINW, :].rearrange("(l s) c -> l (s c)", s=ST),
                    in_=lrev[:],
                )

            # ---- CRF ----
            with (
                tc.tile_pool(name="crf", bufs=2) as fp,
                tc.tile_pool(name="crfc", bufs=1) as fc1,
                tc.tile_pool(name="psc", bufs=2, space="PSUM") as pc,
            ):
                shi_sb = fc1.tile([128, 128], BF16, name="shi")
                nc.sync.dma_start(out=shi_sb[:], in_=shi[:])
                slo_sb = fc1.tile([128, 128], BF16, name="slo")
                nc.sync.dma_start(out=slo_sb[:], in_=slo[:])
                valid_sb = fc1.tile([128, NT], F32, name="valid")
                nc.sync.dma_start(out=valid_sb[:], in_=valid[:])

                uf = fc1.tile([128, CRFW], F32, name="uf")
                nc.sync.dma_start(out=uf[:].rearrange("p (T c) -> p T c", c=C),
                                  in_=bass.AP(logf_d[:].tensor, 0,
                                              [[C, 128], [CST * C, NT], [1, C]]))
                ub = fc1.tile([128, CRFW], F32, name="ub")
                nc.sync.dma_start(out=ub[:].rearrange("p (T c) -> p T c", c=C),
                                  in_=bass.AP(logb_d[:].tensor, 64 * C,
                                              [[C, 128], [CST * C, NT], [1, C]]))
                u = fc1.tile([128, CRFW], F32, name="u")
                nc.vector.tensor_add(u[:], uf[:], ub[:])

                xcur = u
                for it in range(NIT + 1):
                    last = it == NIT
                    e = fp.tile([128, CRFW], F32, name=f"e{it}", tag="e")
                    nc.scalar.activation(e[:], xcur[:], AF.Exp)
                    ssum = fp.tile([128, NT], F32, name=f"ss{it}", tag="ss")
                    nc.vector.reduce_sum(ssum[:], e[:].rearrange("p (T c) -> p T c", c=C),
                                         axis=mybir.AxisListType.X)
                    rv = fp.tile([128, NT], F32, name=f"rv{it}", tag="rv")
                    nc.vector.reciprocal(rv[:], ssum[:])
                    if not last:
                        rvv = fp.tile([128, NT], F32, name=f"rvv{it}", tag="rvv")
                        nc.vector.tensor_mul(rvv[:], rv[:], valid_sb[:])
                        p = fp.tile([128, CRFW], BF16, name=f"p{it}", tag="p")
                        nc.vector.tensor_mul(
                            p[:].rearrange("p (T c) -> p T c", c=C),
                            e[:].rearrange("p (T c) -> p T c", c=C),
                            rvv[:].unsqueeze(2).broadcast_to([128, NT, C]))
                        psc = pc.tile([128, CRFW], F32, name=f"pc{it}", tag="pc")
                        for T in range(NT):
                            nc.tensor.matmul(psc[:, C * T:C * (T + 1)], lhsT=shi_sb[:],
                                             rhs=p[:, C * T:C * (T + 1)],
                                             start=True, stop=False)
                            nc.tensor.matmul(psc[:, C * T:C * (T + 1)], lhsT=slo_sb[:],
                                             rhs=p[:, C * T:C * (T + 1)],
                                             start=False, stop=True)
                        xn = fp.tile([128, CRFW], F32, name=f"x{it}", tag="x")
                        nc.vector.tensor_add(xn[:], u[:], psc[:])
                        xcur = xn
                    else:
                        pout = fp.tile([128, CRFW], F32, name="pout", tag="p")
                        nc.vector.tensor_mul(
                            pout[:].rearrange("p (T c) -> p T c", c=C),
                            e[:].rearrange("p (T c) -> p T c", c=C),
                            rv[:].unsqueeze(2).broadcast_to([128, NT, C]))
                        nc.sync.dma_start(out=out[:], in_=pout[:])

    nc.compile()
    return nc


def _prep(inputs):
    I = {k: np.asarray(v, np.float32) for k, v in inputs.items()}
    x = I["batch"]
    xr = x[::-1]
    bf = ml_dtypes.bfloat16

    Wf = np.concatenate([I["W_ih_f"].T, I["W_hh_f"].T], 0)  # (1024, 2048)
    Wb = np.concatenate([I["W_ih_b"].T, I["W_hh_b"].T], 0)
    biasf = np.broadcast_to((I["b_ih_f"] + I["b_hh_f"])[None, :], (128, G)).copy()
    biasb = np.broadcast_to((I["b_ih_b"] + I["b_hh_b"])[None, :], (128, G)).copy()
    WlinT = I["W_lin"].T  # (1024, 32)

    half = FILT // 2
    dd = np.arange(-half, half + 1, dtype=np.float32)
    kern = np.exp(-(dd * I["inv_smoothness_theta"][0]) ** 2 / 2)
    kern[half] = 0.0
    kern *= I["smoothness_weight"]
    S = np.zeros((128, 128), np.float32)
    for i in range(128):
        for j in range(max(0, i - half), min(128, i + half + 1)):
            if i != j:
                S[i, j] = kern[j - i + half]
    S_hi = S.astype(bf).astype(np.float32)
    S_lo = (S - S_hi).astype(bf)

    shared = dict(
        wf=Wf.reshape(8, 128, G).astype(bf),
        wb=Wb.reshape(8, 128, G).astype(bf),
        biasf=biasf.astype(bf), biasb=biasb.astype(bf),
        wlinf=WlinT[:512].reshape(4, 128, C).astype(bf),
        wlinb=WlinT[512:].reshape(4, 128, C).astype(bf),
        blin=I["b_lin"][None, :].astype(bf),
        ones=np.ones((1, 128), bf),
        ident=np.eye(128, dtype=np.float32).astype(bf),
        rmat=np.eye(128, dtype=np.float32)[::-1].copy(),
        shi=S_hi.astype(bf), slo=S_lo,
    )

    def window(src, W0):
        w = np.zeros((K + WINW, EMB), np.float32)
        lo, hi = W0 - K, W0 + WINW
        slo, shi_ = max(lo, 0), min(hi, SEQ)
        if shi_ > slo:
            w[slo - lo:shi_ - lo] = src[slo:shi_]
        return np.ascontiguousarray(w.T).reshape(4, 128, K + WINW).astype(bf)

    st = np.arange(STEPS)[None, :]
    ll = np.arange(NL)[:, None] * ST
    pp = np.arange(128)[:, None]
    TT = np.arange(NT)[None, :] * CST
    in_maps = []
    for c in range(NCORES):
        Wc = 2048 * c - 32
        Wr = 2048 * (7 - c) - 32
        gpos = Wc + TT + pp
        m = dict(shared)
        m["xtf"] = window(x, Wc)
        m["xtb"] = window(xr, Wr)
        m["mf"] = ((ll + st + Wc - K) >= 0).astype(np.float32)
        m["mb"] = ((ll + st + Wr - K) >= 0).astype(np.float32)
        m["valid"] = ((gpos >= 0) & (gpos < SEQ) & (TT + pp < WINW)).astype(np.float32)
        in_maps.append(m)
    return in_maps


def _run(inputs, trace=False, trace_cores=None):
    if "nc" not in _CACHE:
        _CACHE["nc"] = _build()
    nc = _CACHE["nc"]
    in_maps = _prep(inputs)
    kw = {}
    if trace:
        import types
        try:
            import trn_agent_boot.trn_boot as tb
            hook = tb._ntff_profile_via_ctypes("/opt/axon/libaxon_pjrt.so")
            mod = types.ModuleType("antenv.axon_hooks")
            mod.get_axon_ntff_profile_hook = lambda: hook
            sys.modules.setdefault("antenv.axon_hooks", mod)
        except Exception:
            pass
        kw = dict(trace=True, trace_cores=trace_cores or list(range(NCORES)))
    res = run_bass_kernel_spmd(nc, in_maps, list(range(NCORES)), **kw)
    # decode [128, NT, C] CRF tiles -> window positions.  Tile T covers
    # window positions [CST*T, CST*T+128); rows 25..102 are authoritative
    # (25-deep halo erosion each side), plus tile 0's head rows 0..24.
    wpos = np.arange(32, 32 + 2048)
    TT = np.clip((wpos - 25) // CST, 0, NT - 1)
    pp = wpos - CST * TT
    full = np.zeros((SEQ, C), np.float32)
    for c in range(NCORES):
        o = res.results[c]["out"].reshape(128, NT, C)
        full[2048 * c:2048 * (c + 1)] = o[pp, TT]
    return full, res


def kernel(**inputs):
    full, _ = _run(inputs)
    return full

